# revision 17
# baseline (speedup 1.0000x reference)
"""Discounted cumsum (B,H,S,D)=(8,16,4096,128), gamma per head, scan along S.

Batch-parallel across 8 NeuronCores (1 batch each). IO is int8 fixed-point
both directions (DMA-bound problem; int8 = 2x less traffic than bf16).

Hybrid per-head execution, heads ranked by error amplification
amp = 1/sqrt(1-g^2):

- 6 highest-amp heads -> MATMUL path (needs fine input quantization):
  host quantizes x*32 (clip +-127, ~1.7k outliers get an exact host-side
  decay-tail fix). On device: int8 -> bf16 upcast (exact; 1/32 and the
  output scale 11 are folded into the A-matrix), then the blocked scan as
  fused matmuls: Tb=127 blocks, per-block carry rides the 128th contraction
  row, block sums via 33 N=1 weight-load matmuls, block-level carry scan as
  a 33x33 matmul. PSUM f32 holds y*11; ACT copies straight to int8.

- 10 low-amp heads -> DVE SCAN path: native tensor_tensor_scan
  (state = gamma*state + x, fp32 state, round-to-nearest int8 out).
  Per-head scale S_h = 127/(6.2*amp_h) applied host-side; layout [d, t] so
  the scan runs along the free dim. int8 in -> int8 out, no upcast, no PSUM,
  no copies. gamma comes from a [128,1] column broadcast (stride-0 AP).
"""
import sys

sys.path.insert(0, "/opt/trn_rl_repo")
import ml_dtypes
import numpy as np

BF16 = ml_dtypes.bfloat16
B, H, S, D = 8, 16, 4096, 128
MH = 6           # matmul-path heads (highest amp)
VH = 10          # scan-path heads
MP = MH // 2     # matmul head pairs
TB = 127         # block length along S (127 so carry rides in row 128)
KB = 33          # ceil(S / TB) blocks per head (last block partial)
FD = KB * D      # 4224 free columns per head
PF = 2 * FD      # 8448 free columns per pair tile
SX = 32.0        # matmul-path input scale
SY = 11.0        # matmul-path output scale
AMP_MARGIN = 6.2  # scan-path: S_h = 127/(AMP_MARGIN*amp_h)

_CACHE = {}


def _build(repeat=1, mode="full"):
    import contextlib

    import concourse.bacc as bacc
    import concourse.tile as tile
    from concourse import mybir

    f32 = mybir.dt.float32
    bf16 = mybir.dt.bfloat16
    i8 = mybir.dt.int8

    nc = bacc.Bacc("TRN2", target_bir_lowering=False, debug=False)

    xm_in = nc.declare_dram_parameter("xm", [MP, 128, PF], i8, isOutput=False)
    xs_in = nc.declare_dram_parameter("xs", [VH, 128, S], i8, isOutput=False)
    atg_in = nc.declare_dram_parameter("atg", [128, MH * 128], bf16, isOutput=False)
    w_in = nc.declare_dram_parameter("w", [TB, MH], bf16, isOutput=False)
    abt_in = nc.declare_dram_parameter("abt", [KB, MH * KB], bf16, isOutput=False)
    id_in = nc.declare_dram_parameter("idm", [128, 128], bf16, isOutput=False)
    gc_in = nc.declare_dram_parameter("gcol", [128, 32], f32, isOutput=False)
    ym_out = nc.declare_dram_parameter("ym", [MP, 128, PF], i8, isOutput=True)
    ys_out = nc.declare_dram_parameter("ys", [VH, 128, S], i8, isOutput=True)

    with tile.TileContext(nc) as tc:
        with (
            tc.tile_pool(name="const", bufs=1) as const_pool,
            tc.tile_pool(name="xp", bufs=3) as x_pool,
            tc.tile_pool(name="xb", bufs=3) as xb_pool,
            tc.tile_pool(name="op", bufs=2) as out_pool,
            tc.tile_pool(name="sx", bufs=4) as sx_pool,
            tc.tile_pool(name="sy", bufs=4) as sy_pool,
            tc.tile_pool(name="small", bufs=4) as small_pool,
            tc.tile_pool(name="sstage", bufs=2) as sstage_pool,
            tc.tile_pool(name="stps", bufs=1, space="PSUM") as st_psum,
            tc.tile_pool(name="scps", bufs=1, space="PSUM") as sc_psum,
            tc.tile_pool(name="ybig", bufs=2, space="PSUM") as yb_psum,
        ):
            atg_sb = const_pool.tile([128, MH * 128], bf16)
            w_sb = const_pool.tile([TB, MH], bf16)
            abt_sb = const_pool.tile([KB, MH * KB], bf16)
            id_sb = const_pool.tile([128, 128], bf16)
            gc_sb = const_pool.tile([128, 32], f32)
            nc.sync.dma_start(out=atg_sb[:], in_=atg_in[:])
            nc.sync.dma_start(out=w_sb[:], in_=w_in[:])
            nc.sync.dma_start(out=abt_sb[:], in_=abt_in[:])
            nc.sync.dma_start(out=id_sb[:], in_=id_in[:])
            nc.sync.dma_start(out=gc_sb[:], in_=gc_in[:])
            CL = 2304  # scan chunk length (2048 + 256 burn-in overlap)
            ones_sb = const_pool.tile([128, CL], bf16, name="ones")
            nc.vector.memset(ones_sb[:], 1.0)
            gm = [const_pool.tile([128, CL], bf16, name=f"gm{v}") for v in range(VH)]
            for v in range(VH):
                nc.vector.tensor_scalar_mul(
                    out=gm[v][:], in0=ones_sb[:], scalar1=gc_sb[:, 2 * v : 2 * v + 1]
                )

            xt = [None] * MP     # pair int8 tiles [128, PF]
            xb = [None] * MP     # pair bf16 tiles [128, PF]; row 127 = carries
            yt = [None] * MP     # pair output staging [128, PF] int8
            s32 = [None] * MH    # block sums [KB, D]
            xs = [None] * VH     # scan input tiles [128, S] int8
            ys = [None] * VH     # scan output tiles [128, S] int8
            upcast = mode not in ("computeonly",)

            def m_in(j):
                xt[j] = x_pool.tile([128, PF], i8, name=f"xt{j}", tag="xt")
                nc.sync.dma_start(out=xt[j][:, 0:FD], in_=xm_in[j][:, 0:FD])
                nc.sync.dma_start(out=xt[j][:, FD:PF], in_=xm_in[j][:, FD:PF])

            def s_in(v):
                xs[v] = sx_pool.tile([128, S], i8, name=f"xs{v}", tag="xs")
                nc.sync.dma_start(out=xs[v][:], in_=xs_in[v][:])

            def s_scan(v):
                ys[v] = sy_pool.tile([128, S], i8, name=f"ys{v}", tag="ys")
                # chunk B first (cols 1792..4095; first 256 are burn-in whose
                # decayed-to-zero init error is < g^256 ~ 1e-15), then chunk A
                # (cols 0..2303, exact) overwrites B's burn-in region.
                nc.vector.tensor_tensor_scan(
                    out=ys[v][:, S - CL : S],
                    data0=gm[v][:],
                    data1=xs[v][:, S - CL : S],
                    initial=0.0,
                    op0=mybir.AluOpType.mult,
                    op1=mybir.AluOpType.add,
                )
                nc.vector.tensor_tensor_scan(
                    out=ys[v][:, 0:CL],
                    data0=gm[v][:],
                    data1=xs[v][:, 0:CL],
                    initial=0.0,
                    op0=mybir.AluOpType.mult,
                    op1=mybir.AluOpType.add,
                )

            def s_out(v):
                nc.gpsimd.dma_start(out=ys_out[v][:], in_=ys[v][:])

            st_ps = [None] * MH
            st_sb = [None] * MH

            def m_s_mm(h):
                j, c0 = h // 2, (h % 2) * FD
                if upcast:
                    if h % 2 == 0:
                        xb[j] = xb_pool.tile([128, PF], bf16, name=f"xb{j}", tag="xb")
                    if h % 2 == 1:
                        nc.vector.tensor_copy(
                            out=xb[j][:, c0 : c0 + FD], in_=xt[j][:, c0 : c0 + FD]
                        )
                    else:
                        nc.scalar.copy(
                            out=xb[j][:, c0 : c0 + FD], in_=xt[j][:, c0 : c0 + FD]
                        )
                st_ps[h] = st_psum.tile([128, KB], f32, name="stps", tag="stps")
                for k in range(KB):
                    nc.tensor.matmul(
                        st_ps[h][:, k : k + 1],
                        xb[j][0:TB, c0 + k * D : c0 + (k + 1) * D],
                        w_sb[:, h : h + 1],
                        start=True,
                        stop=True,
                    )

            def m_s_fin(h):
                st_sb[h] = sstage_pool.tile([128, KB], bf16, name="stsb", tag="stsb")
                nc.vector.tensor_copy(out=st_sb[h][:], in_=st_ps[h][:])
                s32_ps = sc_psum.tile([KB, 128], bf16, name="s32p", tag="s32p")
                nc.tensor.transpose(s32_ps[:], st_sb[h][:], id_sb[:])
                s32[h] = small_pool.tile([KB, 128], bf16, name=f"s32{h}", tag="s32")
                nc.scalar.copy(out=s32[h][:], in_=s32_ps[:])

            def m_c(h):
                j, c0 = h // 2, (h % 2) * FD
                c_ps = sc_psum.tile([KB, D], f32, name="cps", tag="cps")
                nc.tensor.matmul(
                    c_ps[:],
                    abt_sb[:, h * KB : (h + 1) * KB],
                    s32[h][:],
                    start=True,
                    stop=True,
                )
                c32 = small_pool.tile([KB, D], bf16, name=f"c32{h}", tag="c32")
                nc.scalar.copy(out=c32[:], in_=c_ps[:])
                # carry DMAs on the scalar queue: c32 is ACT-produced, so the
                # trigger's wait is ~0 there and doesn't block input DMAs
                if mode == "full":
                    dst = xb[j][TB : TB + 1, c0 : c0 + FD]
                else:
                    scr = small_pool.tile([1, FD], bf16, name="scr", tag="scr")
                    dst = scr[0:1, :]
                nc.scalar.dma_start(out=dst[0:1, 0 : 32 * D], in_=c32[0:32, :])
                nc.scalar.dma_start(out=dst[0:1, 32 * D : FD], in_=c32[32:33, :])

            def m_b(h):
                j, c0 = h // 2, (h % 2) * FD
                if h % 2 == 0:
                    yt[j] = out_pool.tile([128, PF], i8, name=f"yt{j}", tag="yt")
                for tt in range(4):
                    cc = c0 + tt * 1024
                    y_ps = yb_psum.tile([128, 1024], f32, name="ybps", tag="ybps")
                    for half in range(2):
                        nc.tensor.matmul(
                            y_ps[:, half * 512 : half * 512 + 512],
                            atg_sb[:, h * 128 : (h + 1) * 128],
                            xb[j][:, cc + half * 512 : cc + half * 512 + 512],
                            start=True,
                            stop=True,
                        )
                    nc.scalar.copy(out=yt[j][:, cc : cc + 1024], in_=y_ps[:])
                cc = c0 + 4096
                y_ps = yb_psum.tile([128, 1024], f32, name="ybps", tag="ybps")
                nc.tensor.matmul(
                    y_ps[:, 0:D],
                    atg_sb[:, h * 128 : (h + 1) * 128],
                    xb[j][:, cc : cc + D],
                    start=True,
                    stop=True,
                )
                nc.scalar.copy(out=yt[j][:, cc : cc + D], in_=y_ps[:, 0:D])
                if mode != "computeonly":
                    if h % 2 == 0:
                        nc.gpsimd.dma_start(out=ym_out[j][:, 0:FD], in_=yt[j][:, 0:FD])
                    else:
                        nc.gpsimd.dma_start(
                            out=ym_out[j][:, FD:PF], in_=yt[j][:, FD:PF]
                        )

            def m_dma_out(j):
                nc.gpsimd.dma_start(out=ym_out[j], in_=xt[j][:])

            def s_dma_out(v):
                nc.gpsimd.dma_start(out=ys_out[v][:], in_=xs[v][:])

            if mode == "computeonly":
                xconst = const_pool.tile([128, PF], bf16)
                nc.vector.memset(xconst[:], 0.125)
                xsconst = const_pool.tile([128, S], i8)
                nc.vector.memset(xsconst[:], 1)

                def m_in(j):  # noqa: F811
                    xb[j] = xconst

                def s_in(v):  # noqa: F811
                    xs[v] = xsconst

            loop = tc.For_i(0, repeat, 1) if repeat > 1 else contextlib.nullcontext()
            with loop:
                if mode == "dmaonly":
                    for i in range(10):
                        if i % 2 == 0 and i < 6:
                            m_in(i // 2)
                        s_in(i)
                        if i % 2 == 1 and i < 7:
                            m_dma_out(i // 2)
                        s_dma_out(i)
                else:
                    do_m = mode != "scanonly"
                    do_s = mode != "monly"
                    for i in range(14):
                        if do_m and i % 2 == 0 and i < 6:
                            m_in(i // 2)
                        if do_s and i < 10:
                            s_in(i)
                        if do_s and 0 <= i - 2 < 10:
                            s_scan(i - 2)
                        if do_m and 0 <= i - 3 < MH:
                            m_s_fin(i - 3)
                        if do_m and 0 <= i - 2 < MH:
                            m_s_mm(i - 2)
                        if do_m and 0 <= i - 4 < MH:
                            m_c(i - 4)
                        if do_m and 0 <= i - 5 < MH:
                            m_b(i - 5)
                        if do_s and mode != "computeonly" and 0 <= i - 4 < 10:
                            s_out(i - 4)

    nc.compile()
    return nc


def _constants(gamma, m_heads, v_heads, sv):
    g = np.asarray(gamma, dtype=np.float64)
    m = np.arange(TB)
    diff = m[:, None] - m[None, :]  # [m, p']
    atg = np.zeros((128, MH * 128), np.float64)
    w = np.zeros((TB, MH), np.float64)
    abt = np.zeros((KB, MH * KB), np.float64)
    k = np.arange(KB)
    kdiff = k[None, :] - k[:, None] - 1  # [j, k] -> k-1-j
    for hi, h in enumerate(m_heads):
        gh = g[h]
        Gn = gh ** TB
        a_h = np.where(diff >= 0, gh ** np.maximum(diff, 0), 0.0)  # [m, p']
        atg[0:TB, hi * 128 : hi * 128 + TB] = a_h.T * (SY / SX)
        atg[TB, hi * 128 : hi * 128 + TB] = gh ** (m + 1) * SY
        w[:, hi] = gh ** (TB - 1 - m) / SX
        abt[:, hi * KB : (hi + 1) * KB] = np.where(
            kdiff >= 0, Gn ** np.maximum(kdiff, 0), 0.0
        )
    idm = np.eye(128, dtype=np.float64)
    gcol = np.zeros((128, 32), np.float64)
    for vi, h in enumerate(v_heads):
        gcol[:, 2 * vi] = g[h]
    return (
        atg.astype(BF16),
        w.astype(BF16),
        abt.astype(BF16),
        idm.astype(BF16),
        gcol.astype(np.float32),
    )


def _head_split(gamma):
    g = np.asarray(gamma, dtype=np.float64)
    amp = 1.0 / np.sqrt(1.0 - np.clip(g, 0, 0.9999) ** 2)
    order = np.argsort(-amp)
    m_heads = order[:MH]
    v_heads = order[MH:]
    sv = 127.0 / (AMP_MARGIN * amp[v_heads])
    return m_heads, v_heads, sv


def _prepare(tensor, gamma):
    """Host-side prep: int8 quantize + pack (matmul pairs + scan slots)."""
    m_heads, v_heads, sv = _head_split(gamma)
    atg, w, abt, idm, gcol = _constants(np.asarray(gamma), m_heads, v_heads, sv)
    xf = np.asarray(tensor, dtype=np.float32)  # [B,H,S,D]
    xq = np.clip(np.round(xf[:, m_heads] * SX), -127, 127).astype(np.int8)
    in_maps = []
    for c in range(B):
        xpad = np.zeros((MH, KB * TB, D), np.int8)
        xpad[:, :S] = xq[c]
        perm = np.ascontiguousarray(
            xpad.reshape(MH, KB, TB, D).transpose(0, 2, 1, 3)
        ).reshape(MH, TB, FD)
        xp = np.zeros((MP, 128, PF), np.int8)
        xp[:, :TB, :FD] = perm[0::2]
        xp[:, :TB, FD:] = perm[1::2]
        xsq = np.empty((VH, 128, S), np.int8)
        for vi, h in enumerate(v_heads):
            xsq[vi] = np.round(xf[c, h] * sv[vi]).astype(np.int8).T
        in_maps.append(
            {"xm": xp, "xs": xsq, "atg": atg, "w": w, "abt": abt, "idm": idm,
             "gcol": gcol}
        )
    return in_maps


def _outlier_fix(y, tensor, gamma):
    """Exact decay-tail correction for host-clipped matmul-path inputs."""
    m_heads, _, _ = _head_split(gamma)
    xf = np.asarray(tensor, dtype=np.float32)[:, m_heads]
    xs_ = xf * SX
    xr = np.round(xs_)
    mask = np.abs(xr) > 127
    if not mask.any():
        return y
    resid = (xs_ - np.clip(xr, -127, 127)) / SX
    g = np.asarray(gamma, dtype=np.float64)[m_heads]
    powg = (g[:, None] ** np.arange(S)[None, :]).astype(np.float32)
    bs, hs, ts, ds = np.nonzero(mask)
    for b, hi, t, d in zip(bs, hs, ts, ds):
        y[b, m_heads[hi], t:, d] += resid[b, hi, t, d] * powg[hi, : S - t]
    return y


def _postprocess(res, gamma):
    """Device outputs -> [H, S, D] f32 for one core."""
    m_heads, v_heads, sv = _head_split(gamma)
    y = np.empty((H, S, D), np.float32)
    ym = res["ym"]  # [MP, 128, PF] int8
    arr = np.stack([ym[:, :TB, :FD], ym[:, :TB, FD:]], axis=1)
    ymf = (
        arr.astype(np.float32)
        .reshape(MH, TB, KB, D)
        .transpose(0, 2, 1, 3)
        .reshape(MH, KB * TB, D)[:, :S]
        / SY
    )
    for hi, h in enumerate(m_heads):
        y[h] = ymf[hi]
    ysd = res["ys"]  # [VH, 128, S] int8
    for vi, h in enumerate(v_heads):
        y[h] = (ysd[vi].astype(np.float32) / sv[vi]).T
    return y


def _fast_callable(nc):
    """Cached jitted shard_map callable (avoids per-call retrace)."""
    import jax
    from jax.experimental.shard_map import shard_map
    from jax.sharding import Mesh, NamedSharding, PartitionSpec
    from concourse import bass2jax, mybir

    bass2jax.install_neuronx_cc_hook()
    partition_name = nc.partition_id_tensor.name if nc.partition_id_tensor else None
    in_names, out_names, out_avals, zero_outs = [], [], [], []
    for alloc in nc.m.functions[0].allocations:
        if not isinstance(alloc, mybir.MemoryLocationSet):
            continue
        name = alloc.memorylocations[0].name
        if alloc.kind == "ExternalInput":
            if name != partition_name:
                in_names.append(name)
        elif alloc.kind == "ExternalOutput":
            shape = tuple(alloc.tensor_shape)
            dtype = mybir.dt.np(alloc.dtype)
            out_avals.append(jax.core.ShapedArray(shape, dtype))
            out_names.append(name)
            zero_outs.append(np.zeros(shape, dtype))
    n_params = len(in_names)
    all_in = list(in_names) + list(out_names)
    if partition_name is not None:
        all_in.append(partition_name)

    def _body(*args):
        operands = list(args)
        if partition_name is not None:
            operands.append(bass2jax.partition_id_tensor())
        return tuple(
            bass2jax._bass_exec_p.bind(
                *operands,
                out_avals=tuple(out_avals),
                in_names=tuple(all_in),
                out_names=tuple(out_names),
                lowering_input_output_aliases=(),
                sim_require_finite=True,
                sim_require_nnan=True,
                nc=nc,
            )
        )

    devices = jax.devices()[:B]
    mesh = Mesh(np.asarray(devices), ("core",))
    specs = (PartitionSpec("core"),)
    f = jax.jit(
        shard_map(
            _body,
            mesh=mesh,
            in_specs=specs * (n_params + len(out_names)),
            out_specs=specs * len(out_names),
            check_rep=False,
        ),
        keep_unused=True,
    )
    sharding = NamedSharding(mesh, PartitionSpec("core"))
    dev_zero = [
        jax.device_put(np.zeros((B * z.shape[0], *z.shape[1:]), z.dtype), sharding)
        for z in zero_outs
    ]
    return f, in_names, out_names, out_avals, sharding, dev_zero


def _run_fast(nc, in_maps):
    import jax

    if "fast" not in _CACHE:
        _CACHE["fast"] = _fast_callable(nc)
    f, in_names, out_names, out_avals, sharding, dev_zero = _CACHE["fast"]
    concat_in = [
        jax.device_put(
            np.concatenate([np.asarray(m[nm]) for m in in_maps], axis=0), sharding
        )
        for nm in in_names
    ]
    outs = f(*concat_in, *dev_zero)
    return [
        {
            nm: np.asarray(outs[i]).reshape(B, *out_avals[i].shape)[c]
            for i, nm in enumerate(out_names)
        }
        for c in range(B)
    ]


def _run(tensor, gamma, trace=False, repeat=1):
    from concourse.bass_utils import run_bass_kernel_spmd

    key = f"nc{repeat}"
    if key not in _CACHE:
        _CACHE[key] = _build(repeat)
    nc = _CACHE[key]

    in_maps = _prepare(tensor, gamma)
    if repeat == 1 and not trace:
        try:
            results = _run_fast(nc, in_maps)
            y = np.stack(
                [_postprocess(results[c], gamma) for c in range(B)], axis=0
            )
            return _outlier_fix(y, tensor, gamma), None
        except Exception:
            pass  # fall back to the reference path below
    res = run_bass_kernel_spmd(nc, in_maps, core_ids=list(range(B)), trace=trace)
    y = np.stack([_postprocess(res.results[c], gamma) for c in range(B)], axis=0)
    return _outlier_fix(y, tensor, gamma), res


def kernel(tensor, gamma):
    try:
        y, _ = _run(tensor, gamma)
    except Exception:
        # transient device/pool errors: clear cached state and retry once
        _CACHE.clear()
        y, _ = _run(tensor, gamma)
    return y


# revision 18
# speedup vs baseline: 2.6174x; 2.6174x over previous
"""Discounted cumsum (B,H,S,D)=(8,16,4096,128), gamma per head, scan along S.

Batch-parallel across 8 NeuronCores (1 batch each). IO is int8 fixed-point
both directions (DMA-bound problem; int8 = 2x less traffic than bf16).

Hybrid per-head execution, heads ranked by error amplification
amp = 1/sqrt(1-g^2):

- 6 highest-amp heads -> MATMUL path (needs fine input quantization):
  host quantizes x*32 (clip +-127, ~1.7k outliers get an exact host-side
  decay-tail fix). On device: int8 -> bf16 upcast (exact; 1/32 and the
  output scale 11 are folded into the A-matrix), then the blocked scan as
  fused matmuls: Tb=127 blocks, per-block carry rides the 128th contraction
  row, block sums via 33 N=1 weight-load matmuls, block-level carry scan as
  a 33x33 matmul. PSUM f32 holds y*11; ACT copies straight to int8.

- 10 low-amp heads -> DVE SCAN path: native tensor_tensor_scan
  (state = gamma*state + x, fp32 state, round-to-nearest int8 out).
  Per-head scale S_h = 127/(6.2*amp_h) applied host-side; layout [d, t] so
  the scan runs along the free dim. int8 in -> int8 out, no upcast, no PSUM,
  no copies. gamma comes from a [128,1] column broadcast (stride-0 AP).
"""
import sys

sys.path.insert(0, "/opt/trn_rl_repo")
import ml_dtypes
import numpy as np

BF16 = ml_dtypes.bfloat16
B, H, S, D = 8, 16, 4096, 128
MH = 6           # matmul-path heads (highest amp)
VH = 10          # scan-path heads
MP = MH // 2     # matmul head pairs
TB = 127         # block length along S (127 so carry rides in row 128)
KB = 33          # ceil(S / TB) blocks per head (last block partial)
FD = KB * D      # 4224 free columns per head
PF = 2 * FD      # 8448 free columns per pair tile
SX = 32.0        # matmul-path input scale
SY = 11.0        # matmul-path output scale
AMP_MARGIN = 6.2  # scan-path: S_h = 127/(AMP_MARGIN*amp_h)

_CACHE = {}


def _build(repeat=1, mode="full"):
    import contextlib

    import concourse.bacc as bacc
    import concourse.tile as tile
    from concourse import mybir

    f32 = mybir.dt.float32
    bf16 = mybir.dt.bfloat16
    i8 = mybir.dt.int8

    nc = bacc.Bacc("TRN2", target_bir_lowering=False, debug=False)

    xm_in = nc.declare_dram_parameter("xm", [MP, 128, PF], i8, isOutput=False)
    xs_in = nc.declare_dram_parameter("xs", [VH, 128, S], i8, isOutput=False)
    atg_in = nc.declare_dram_parameter("atg", [128, MH * 128], bf16, isOutput=False)
    w_in = nc.declare_dram_parameter("w", [TB, MH], bf16, isOutput=False)
    abt_in = nc.declare_dram_parameter("abt", [KB, MH * KB], bf16, isOutput=False)
    id_in = nc.declare_dram_parameter("idm", [128, 128], bf16, isOutput=False)
    gc_in = nc.declare_dram_parameter("gcol", [128, 32], f32, isOutput=False)
    ym_out = nc.declare_dram_parameter("ym", [MP, 128, PF], i8, isOutput=True)
    ys_out = nc.declare_dram_parameter("ys", [VH, 128, S], i8, isOutput=True)

    with tile.TileContext(nc) as tc:
        with (
            tc.tile_pool(name="const", bufs=1) as const_pool,
            tc.tile_pool(name="xp", bufs=3) as x_pool,
            tc.tile_pool(name="xb", bufs=3) as xb_pool,
            tc.tile_pool(name="op", bufs=2) as out_pool,
            tc.tile_pool(name="sx", bufs=4) as sx_pool,
            tc.tile_pool(name="sy", bufs=4) as sy_pool,
            tc.tile_pool(name="small", bufs=4) as small_pool,
            tc.tile_pool(name="sstage", bufs=2) as sstage_pool,
            tc.tile_pool(name="stps", bufs=1, space="PSUM") as st_psum,
            tc.tile_pool(name="scps", bufs=1, space="PSUM") as sc_psum,
            tc.tile_pool(name="ybig", bufs=2, space="PSUM") as yb_psum,
        ):
            atg_sb = const_pool.tile([128, MH * 128], bf16)
            w_sb = const_pool.tile([TB, MH], bf16)
            abt_sb = const_pool.tile([KB, MH * KB], bf16)
            id_sb = const_pool.tile([128, 128], bf16)
            gc_sb = const_pool.tile([128, 32], f32)
            nc.sync.dma_start(out=atg_sb[:], in_=atg_in[:])
            nc.sync.dma_start(out=w_sb[:], in_=w_in[:])
            nc.sync.dma_start(out=abt_sb[:], in_=abt_in[:])
            nc.sync.dma_start(out=id_sb[:], in_=id_in[:])
            nc.sync.dma_start(out=gc_sb[:], in_=gc_in[:])
            CL = 2048  # scan chunk length (512B-aligned for DVE fast mode)
            ones_sb = const_pool.tile([128, CL], bf16, name="ones")
            nc.vector.memset(ones_sb[:], 1.0)
            gm = [const_pool.tile([128, CL], bf16, name=f"gm{v}") for v in range(VH)]
            for v in range(VH):
                nc.vector.tensor_scalar_mul(
                    out=gm[v][:], in0=ones_sb[:], scalar1=gc_sb[:, 2 * v : 2 * v + 1]
                )

            xt = [None] * MP     # pair int8 tiles [128, PF]
            xb = [None] * MP     # pair bf16 tiles [128, PF]; row 127 = carries
            yt = [None] * MP     # pair output staging [128, PF] int8
            s32 = [None] * MH    # block sums [KB, D]
            xs = [None] * VH     # scan input tiles [128, S] int8
            ys = [None] * VH     # scan output tiles [128, S] int8
            upcast = mode not in ("computeonly",)

            def m_in(j):
                xt[j] = x_pool.tile([128, PF], i8, name=f"xt{j}", tag="xt")
                nc.sync.dma_start(out=xt[j][:, 0:FD], in_=xm_in[j][:, 0:FD])
                nc.sync.dma_start(out=xt[j][:, FD:PF], in_=xm_in[j][:, FD:PF])

            def s_in(v):
                xs[v] = sx_pool.tile([128, S], i8, name=f"xs{v}", tag="xs")
                nc.sync.dma_start(out=xs[v][:], in_=xs_in[v][:])

            def s_scan(v):
                ys[v] = sy_pool.tile([128, S], i8, name=f"ys{v}", tag="ys")
                # two 512B-aligned chunks; chunk B chains off A's last column
                # (int8-quantized carry: adds <= 0.5/S_h decayed error)
                nc.vector.tensor_tensor_scan(
                    out=ys[v][:, 0:CL],
                    data0=gm[v][:],
                    data1=xs[v][:, 0:CL],
                    initial=0.0,
                    op0=mybir.AluOpType.mult,
                    op1=mybir.AluOpType.add,
                )
                nc.vector.tensor_tensor_scan(
                    out=ys[v][:, CL:S],
                    data0=gm[v][:],
                    data1=xs[v][:, CL:S],
                    initial=ys[v][:, CL - 1 : CL],
                    op0=mybir.AluOpType.mult,
                    op1=mybir.AluOpType.add,
                )

            def s_out(v):
                nc.gpsimd.dma_start(out=ys_out[v][:], in_=ys[v][:])

            st_ps = [None] * MH
            st_sb = [None] * MH

            def m_s_mm(h):
                j, c0 = h // 2, (h % 2) * FD
                if upcast:
                    if h % 2 == 0:
                        xb[j] = xb_pool.tile([128, PF], bf16, name=f"xb{j}", tag="xb")
                    if h % 2 == 1:
                        nc.vector.tensor_copy(
                            out=xb[j][:, c0 : c0 + FD], in_=xt[j][:, c0 : c0 + FD]
                        )
                    else:
                        nc.scalar.copy(
                            out=xb[j][:, c0 : c0 + FD], in_=xt[j][:, c0 : c0 + FD]
                        )
                st_ps[h] = st_psum.tile([128, KB], f32, name="stps", tag="stps")
                for k in range(KB):
                    nc.tensor.matmul(
                        st_ps[h][:, k : k + 1],
                        xb[j][0:TB, c0 + k * D : c0 + (k + 1) * D],
                        w_sb[:, h : h + 1],
                        start=True,
                        stop=True,
                    )

            def m_s_fin(h):
                st_sb[h] = sstage_pool.tile([128, KB], bf16, name="stsb", tag="stsb")
                nc.vector.tensor_copy(out=st_sb[h][:], in_=st_ps[h][:])
                s32_ps = sc_psum.tile([KB, 128], bf16, name="s32p", tag="s32p")
                nc.tensor.transpose(s32_ps[:], st_sb[h][:], id_sb[:])
                s32[h] = small_pool.tile([KB, 128], bf16, name=f"s32{h}", tag="s32")
                nc.scalar.copy(out=s32[h][:], in_=s32_ps[:])

            def m_c(h):
                j, c0 = h // 2, (h % 2) * FD
                c_ps = sc_psum.tile([KB, D], f32, name="cps", tag="cps")
                nc.tensor.matmul(
                    c_ps[:],
                    abt_sb[:, h * KB : (h + 1) * KB],
                    s32[h][:],
                    start=True,
                    stop=True,
                )
                c32 = small_pool.tile([KB, D], bf16, name=f"c32{h}", tag="c32")
                nc.scalar.copy(out=c32[:], in_=c_ps[:])
                # carry DMAs on the scalar queue: c32 is ACT-produced, so the
                # trigger's wait is ~0 there and doesn't block input DMAs
                if mode == "full":
                    dst = xb[j][TB : TB + 1, c0 : c0 + FD]
                else:
                    scr = small_pool.tile([1, FD], bf16, name="scr", tag="scr")
                    dst = scr[0:1, :]
                nc.scalar.dma_start(out=dst[0:1, 0 : 32 * D], in_=c32[0:32, :])
                nc.scalar.dma_start(out=dst[0:1, 32 * D : FD], in_=c32[32:33, :])

            def m_b(h):
                j, c0 = h // 2, (h % 2) * FD
                if h % 2 == 0:
                    yt[j] = out_pool.tile([128, PF], i8, name=f"yt{j}", tag="yt")
                for tt in range(4):
                    cc = c0 + tt * 1024
                    y_ps = yb_psum.tile([128, 1024], f32, name="ybps", tag="ybps")
                    for half in range(2):
                        nc.tensor.matmul(
                            y_ps[:, half * 512 : half * 512 + 512],
                            atg_sb[:, h * 128 : (h + 1) * 128],
                            xb[j][:, cc + half * 512 : cc + half * 512 + 512],
                            start=True,
                            stop=True,
                        )
                    nc.scalar.copy(out=yt[j][:, cc : cc + 1024], in_=y_ps[:])
                cc = c0 + 4096
                y_ps = yb_psum.tile([128, 1024], f32, name="ybps", tag="ybps")
                nc.tensor.matmul(
                    y_ps[:, 0:D],
                    atg_sb[:, h * 128 : (h + 1) * 128],
                    xb[j][:, cc : cc + D],
                    start=True,
                    stop=True,
                )
                nc.scalar.copy(out=yt[j][:, cc : cc + D], in_=y_ps[:, 0:D])
                if mode != "computeonly":
                    if h % 2 == 0:
                        nc.gpsimd.dma_start(out=ym_out[j][:, 0:FD], in_=yt[j][:, 0:FD])
                    else:
                        nc.gpsimd.dma_start(
                            out=ym_out[j][:, FD:PF], in_=yt[j][:, FD:PF]
                        )

            def m_dma_out(j):
                nc.gpsimd.dma_start(out=ym_out[j], in_=xt[j][:])

            def s_dma_out(v):
                nc.gpsimd.dma_start(out=ys_out[v][:], in_=xs[v][:])

            if mode == "computeonly":
                xconst = const_pool.tile([128, PF], bf16)
                nc.vector.memset(xconst[:], 0.125)
                xsconst = const_pool.tile([128, S], i8)
                nc.vector.memset(xsconst[:], 1)

                def m_in(j):  # noqa: F811
                    xb[j] = xconst

                def s_in(v):  # noqa: F811
                    xs[v] = xsconst

            loop = tc.For_i(0, repeat, 1) if repeat > 1 else contextlib.nullcontext()
            with loop:
                if mode == "dmaonly":
                    for i in range(10):
                        if i % 2 == 0 and i < 6:
                            m_in(i // 2)
                        s_in(i)
                        if i % 2 == 1 and i < 7:
                            m_dma_out(i // 2)
                        s_dma_out(i)
                else:
                    do_m = mode != "scanonly"
                    do_s = mode != "monly"
                    for i in range(14):
                        if do_m and i % 2 == 0 and i < 6:
                            m_in(i // 2)
                        if do_s and i < 10:
                            s_in(i)
                        if do_s and 0 <= i - 2 < 10:
                            s_scan(i - 2)
                        if do_m and 0 <= i - 3 < MH:
                            m_s_fin(i - 3)
                        if do_m and 0 <= i - 2 < MH:
                            m_s_mm(i - 2)
                        if do_m and 0 <= i - 4 < MH:
                            m_c(i - 4)
                        if do_m and 0 <= i - 5 < MH:
                            m_b(i - 5)
                        if do_s and mode != "computeonly" and 0 <= i - 4 < 10:
                            s_out(i - 4)

    nc.compile()
    return nc


def _constants(gamma, m_heads, v_heads, sv):
    g = np.asarray(gamma, dtype=np.float64)
    m = np.arange(TB)
    diff = m[:, None] - m[None, :]  # [m, p']
    atg = np.zeros((128, MH * 128), np.float64)
    w = np.zeros((TB, MH), np.float64)
    abt = np.zeros((KB, MH * KB), np.float64)
    k = np.arange(KB)
    kdiff = k[None, :] - k[:, None] - 1  # [j, k] -> k-1-j
    for hi, h in enumerate(m_heads):
        gh = g[h]
        Gn = gh ** TB
        a_h = np.where(diff >= 0, gh ** np.maximum(diff, 0), 0.0)  # [m, p']
        atg[0:TB, hi * 128 : hi * 128 + TB] = a_h.T * (SY / SX)
        atg[TB, hi * 128 : hi * 128 + TB] = gh ** (m + 1) * SY
        w[:, hi] = gh ** (TB - 1 - m) / SX
        abt[:, hi * KB : (hi + 1) * KB] = np.where(
            kdiff >= 0, Gn ** np.maximum(kdiff, 0), 0.0
        )
    idm = np.eye(128, dtype=np.float64)
    gcol = np.zeros((128, 32), np.float64)
    for vi, h in enumerate(v_heads):
        gcol[:, 2 * vi] = g[h]
    return (
        atg.astype(BF16),
        w.astype(BF16),
        abt.astype(BF16),
        idm.astype(BF16),
        gcol.astype(np.float32),
    )


def _head_split(gamma):
    g = np.asarray(gamma, dtype=np.float64)
    amp = 1.0 / np.sqrt(1.0 - np.clip(g, 0, 0.9999) ** 2)
    order = np.argsort(-amp)
    m_heads = order[:MH]
    v_heads = order[MH:]
    sv = 127.0 / (AMP_MARGIN * amp[v_heads])
    return m_heads, v_heads, sv


def _prepare(tensor, gamma):
    """Host-side prep: int8 quantize + pack (matmul pairs + scan slots)."""
    m_heads, v_heads, sv = _head_split(gamma)
    atg, w, abt, idm, gcol = _constants(np.asarray(gamma), m_heads, v_heads, sv)
    xf = np.asarray(tensor, dtype=np.float32)  # [B,H,S,D]
    xq = np.clip(np.round(xf[:, m_heads] * SX), -127, 127).astype(np.int8)
    in_maps = []
    for c in range(B):
        xpad = np.zeros((MH, KB * TB, D), np.int8)
        xpad[:, :S] = xq[c]
        perm = np.ascontiguousarray(
            xpad.reshape(MH, KB, TB, D).transpose(0, 2, 1, 3)
        ).reshape(MH, TB, FD)
        xp = np.zeros((MP, 128, PF), np.int8)
        xp[:, :TB, :FD] = perm[0::2]
        xp[:, :TB, FD:] = perm[1::2]
        xsq = np.empty((VH, 128, S), np.int8)
        for vi, h in enumerate(v_heads):
            xsq[vi] = np.round(xf[c, h] * sv[vi]).astype(np.int8).T
        in_maps.append(
            {"xm": xp, "xs": xsq, "atg": atg, "w": w, "abt": abt, "idm": idm,
             "gcol": gcol}
        )
    return in_maps


def _outlier_fix(y, tensor, gamma):
    """Exact decay-tail correction for host-clipped matmul-path inputs."""
    m_heads, _, _ = _head_split(gamma)
    xf = np.asarray(tensor, dtype=np.float32)[:, m_heads]
    xs_ = xf * SX
    xr = np.round(xs_)
    mask = np.abs(xr) > 127
    if not mask.any():
        return y
    resid = (xs_ - np.clip(xr, -127, 127)) / SX
    g = np.asarray(gamma, dtype=np.float64)[m_heads]
    powg = (g[:, None] ** np.arange(S)[None, :]).astype(np.float32)
    bs, hs, ts, ds = np.nonzero(mask)
    for b, hi, t, d in zip(bs, hs, ts, ds):
        y[b, m_heads[hi], t:, d] += resid[b, hi, t, d] * powg[hi, : S - t]
    return y


def _postprocess(res, gamma):
    """Device outputs -> [H, S, D] f32 for one core."""
    m_heads, v_heads, sv = _head_split(gamma)
    y = np.empty((H, S, D), np.float32)
    ym = res["ym"]  # [MP, 128, PF] int8
    arr = np.stack([ym[:, :TB, :FD], ym[:, :TB, FD:]], axis=1)
    ymf = (
        arr.astype(np.float32)
        .reshape(MH, TB, KB, D)
        .transpose(0, 2, 1, 3)
        .reshape(MH, KB * TB, D)[:, :S]
        / SY
    )
    for hi, h in enumerate(m_heads):
        y[h] = ymf[hi]
    ysd = res["ys"]  # [VH, 128, S] int8
    for vi, h in enumerate(v_heads):
        y[h] = (ysd[vi].astype(np.float32) / sv[vi]).T
    return y


def _fast_callable(nc):
    """Cached jitted shard_map callable (avoids per-call retrace)."""
    import jax
    from jax.experimental.shard_map import shard_map
    from jax.sharding import Mesh, NamedSharding, PartitionSpec
    from concourse import bass2jax, mybir

    bass2jax.install_neuronx_cc_hook()
    partition_name = nc.partition_id_tensor.name if nc.partition_id_tensor else None
    in_names, out_names, out_avals, zero_outs = [], [], [], []
    for alloc in nc.m.functions[0].allocations:
        if not isinstance(alloc, mybir.MemoryLocationSet):
            continue
        name = alloc.memorylocations[0].name
        if alloc.kind == "ExternalInput":
            if name != partition_name:
                in_names.append(name)
        elif alloc.kind == "ExternalOutput":
            shape = tuple(alloc.tensor_shape)
            dtype = mybir.dt.np(alloc.dtype)
            out_avals.append(jax.core.ShapedArray(shape, dtype))
            out_names.append(name)
            zero_outs.append(np.zeros(shape, dtype))
    n_params = len(in_names)
    all_in = list(in_names) + list(out_names)
    if partition_name is not None:
        all_in.append(partition_name)

    def _body(*args):
        operands = list(args)
        if partition_name is not None:
            operands.append(bass2jax.partition_id_tensor())
        return tuple(
            bass2jax._bass_exec_p.bind(
                *operands,
                out_avals=tuple(out_avals),
                in_names=tuple(all_in),
                out_names=tuple(out_names),
                lowering_input_output_aliases=(),
                sim_require_finite=True,
                sim_require_nnan=True,
                nc=nc,
            )
        )

    devices = jax.devices()[:B]
    mesh = Mesh(np.asarray(devices), ("core",))
    specs = (PartitionSpec("core"),)
    f = jax.jit(
        shard_map(
            _body,
            mesh=mesh,
            in_specs=specs * (n_params + len(out_names)),
            out_specs=specs * len(out_names),
            check_rep=False,
        ),
        keep_unused=True,
    )
    sharding = NamedSharding(mesh, PartitionSpec("core"))
    dev_zero = [
        jax.device_put(np.zeros((B * z.shape[0], *z.shape[1:]), z.dtype), sharding)
        for z in zero_outs
    ]
    return f, in_names, out_names, out_avals, sharding, dev_zero


def _run_fast(nc, in_maps):
    import jax

    if "fast" not in _CACHE:
        _CACHE["fast"] = _fast_callable(nc)
    f, in_names, out_names, out_avals, sharding, dev_zero = _CACHE["fast"]
    concat_in = [
        jax.device_put(
            np.concatenate([np.asarray(m[nm]) for m in in_maps], axis=0), sharding
        )
        for nm in in_names
    ]
    outs = f(*concat_in, *dev_zero)
    return [
        {
            nm: np.asarray(outs[i]).reshape(B, *out_avals[i].shape)[c]
            for i, nm in enumerate(out_names)
        }
        for c in range(B)
    ]


def _run(tensor, gamma, trace=False, repeat=1):
    from concourse.bass_utils import run_bass_kernel_spmd

    key = f"nc{repeat}"
    if key not in _CACHE:
        _CACHE[key] = _build(repeat)
    nc = _CACHE[key]

    in_maps = _prepare(tensor, gamma)
    if repeat == 1 and not trace:
        try:
            results = _run_fast(nc, in_maps)
            y = np.stack(
                [_postprocess(results[c], gamma) for c in range(B)], axis=0
            )
            return _outlier_fix(y, tensor, gamma), None
        except Exception:
            pass  # fall back to the reference path below
    res = run_bass_kernel_spmd(nc, in_maps, core_ids=list(range(B)), trace=trace)
    y = np.stack([_postprocess(res.results[c], gamma) for c in range(B)], axis=0)
    return _outlier_fix(y, tensor, gamma), res


def kernel(tensor, gamma):
    try:
        y, _ = _run(tensor, gamma)
    except Exception:
        # transient device/pool errors: clear cached state and retry once
        _CACHE.clear()
        y, _ = _run(tensor, gamma)
    return y
